# revision 1
# baseline (speedup 1.0000x reference)
"""ColorDenseCRFLoss on 8 Trainium2 NeuronCores — ACT-bound redesign.

Math: loss = -W/N * sum_n sum_ij K_ij S_ij, where for each image n
  K_ij = exp(-0.5*||f_i - f_j||^2)   (f = nearest-downsampled RGB / 15, P=4096 pts)
  S_ij = sum_k seg_k,i seg_k,j       (seg = bilinear-downsampled softmax, K=21)
Bilinear downsample at exactly 2x == 2x2 average pooling; nearest == stride-2.

Sharding: 2 cores per image (batch N=4 -> 8 cores). Symmetry via circulant
blocks: core h of image n owns row-blocks v=0..15 (of its rotated frame,
rotation 2048*h points) with column window d=0..16 (2176 cols); d=0 and d=16
columns are weighted 1/2 (folded on-device into the seg windows) and the
grand total is doubled.

The kernel is scalar-engine(exp)-bound: ~38us of EXP at 1 elem/cycle/lane.
Everything else is scheduled to keep ACT streaming back-to-back:
 - G = A'^T B' via PE with 4x row-tiling: the K=15 augmented-feature matmuls
   of the four row-blocks of a group run on PE sub-tiles T0/T4/T8/T12
   concurrently (operands replicated at partition offsets 0/32/64/96), so PE
   G cost is ~4x down and stays far below ACT even HAM-cold.
 - EXP reads [128,1024] PSUM pairs (two row-blocks per instruction).
 - AS = segT^T K via PE 4x column-tiling into one stacked PSUM tile
   (bands at partition offsets 0/32/64/96), one fused DVE
   scalar_tensor_tensor multiply-reduce per 512-col chunk.
 - seg 2x2-sum-pool + transposes + window staging all run in the shadow of
   the first EXP batches; no HAM warm-up matmuls, no idle ACT gaps.
"""

import sys

for _p in ("/opt/trn_rl_repo",):
    if _p not in sys.path:
        sys.path.insert(0, _p)

import numpy as np
import ml_dtypes

import bass_rust
import concourse.bass as bass
import concourse.mybir as mybir
from concourse.tile import TileContext
from concourse.bass_utils import run_bass_kernel_spmd

F32 = mybir.dt.float32
BF16 = mybir.dt.bfloat16

WEIGHT = 1e-7
SIGMA_RGB = 15.0
N_IMG = 4
P = 4096          # 64*64 points per image
WIN = 17 * 128    # d = 0..16 column window (2176)

_CACHED = {}


def _build_nc():
    nc = bass.Bass(trn_type="TRN2", target_bir_lowering=False, debug=False)
    seg_d = nc.dram_tensor("segr", [21, 128, 128], F32, kind="ExternalInput")
    a_d = nc.dram_tensor("abf", [60, 512], BF16, kind="ExternalInput")
    b_d = nc.dram_tensor("bbf", [60, 4096], BF16, kind="ExternalInput")
    id_d = nc.dram_tensor("ident", [32, 32], BF16, kind="ExternalInput")
    out_d = nc.dram_tensor("acc", [128, 24], F32, kind="ExternalOutput")

    EXP = mybir.ActivationFunctionType.Exp
    MULT = mybir.AluOpType.mult

    with TileContext(nc) as tc:
        with (
            tc.tile_pool(name="const", bufs=1) as constp,
            tc.tile_pool(name="pre", bufs=1) as prep,
            tc.tile_pool(name="kbuf", bufs=28) as kp,
            tc.tile_pool(name="scr", bufs=2) as scp,
            tc.tile_pool(name="pg", bufs=3, space="PSUM") as pg,
            tc.tile_pool(name="stkp", bufs=2, space="PSUM") as stkp,
        ):
            bias0 = constp.tile([128, 1], F32, tag="bias0")
            nc.vector.memset(bias0[:], 0.0)

            # Warm-up exp with no other deps: the one-time ACT table load
            # (~2.7us) starts immediately instead of waiting on the first G.
            warm = constp.tile([128, 1], F32, tag="warm")
            nc.scalar.activation(warm[:], bias0[:], EXP, bias=bias0[:])

            # Inputs. abf/bbf replicated per partition quadrant 32q so the
            # four row-blocks of a group run on PE row-tiles concurrently.
            # Per-quadrant loads. Same-tile dma_starts serialize (WAW), so
            # load q0/q2 first: the pair-A G matmuls need only those, and
            # g_batch(0) consumes all pair-A chunks before pair-B.
            abf = constp.tile([128, 512], BF16, tag="abf")
            bbf = constp.tile([128, 4096], BF16, tag="bbf")
            for q in (0, 2, 1, 3):
                nc.sync.dma_start(
                    abf[32 * q : 32 * q + 15, :], a_d.ap()[15 * q : 15 * q + 15, :]
                )
            for q in (0, 2, 1, 3):
                nc.sync.dma_start(
                    bbf[32 * q : 32 * q + 15, :], b_d.ap()[15 * q : 15 * q + 15, :]
                )
            ident = constp.tile([32, 32], BF16, tag="ident")
            nc.sync.dma_start(ident[:], id_d.ap())

            # seg pipeline: contiguous load [84,4096] f32, strided adds for
            # the 2x2 sum-pool (/16 folded into host const), one SBUF->SBUF
            # re-layout to [21,4096] bf16.
            segfull = prep.tile([84, 4096], F32, tag="segfull")
            nc.sync.dma_start(segfull[:], seg_d.ap())
            sf = segfull[:].rearrange("p (y x) -> p y x", x=128)  # [84, 32, 128]
            ypool = prep.tile([84, 2048], F32, tag="ypool")
            yp = ypool[:].rearrange("p (y x) -> p y x", x=128)    # [84, 16, 128]
            nc.vector.tensor_add(yp, sf[:, 0::2, :], sf[:, 1::2, :])
            xin = ypool[:].rearrange("p (y x) -> p y x", x=2)     # [84, 1024, 2]
            pooled = prep.tile([84, 1024], BF16, tag="poolf")
            nc.vector.tensor_add(pooled[:], xin[:, :, 0], xin[:, :, 1])
            seg_sb = prep.tile([21, 4096], BF16, tag="segsb")
            nc.sync.dma_start(seg_sb[:], pooled[:])

            # segT [128, 16*32]: column band 32v holds seg[:, v*128:+128]^T
            # (cols 21..31 zero) -- the M=32 AS weights.
            segT = prep.tile([128, 512], BF16, tag="segT")
            nc.gpsimd.memset(segT[:], 0.0)

            # Stacked seg windows: segstk[g][32q+k, :] = seg[k, win(4g+q)]
            # (rows 21..31 of each band zeroed). d=0 and d=16 column ranges
            # pre-scaled by 1/2 on-device.
            segstk = []
            for g in range(4):
                t = prep.tile([128, WIN], BF16, tag=f"segstk{g}")
                nc.gpsimd.memset(t[:], 0.0)
                segstk.append(t)
            for v in range(16):
                g, q = v // 4, v % 4
                nc.sync.dma_start(
                    segstk[g][32 * q : 32 * q + 21, :],
                    seg_sb[:, 128 * v : 128 * v + WIN],
                )

            accT = constp.tile([128, 24], F32, tag="accT")

            def pslice(t, lo, n, c0, c1):
                # [lo:lo+n, c0:c1] partition+col slice; base 96 must be
                # expressed as a double-slice (AP base_partition rejects 96).
                if lo >= 96:
                    return t[64:128, c0:c1][lo - 64 : lo - 64 + n, :]
                return t[lo : lo + n, c0:c1]

            # kt tiles per group: 4x ktA (q0|q2), 4x ktB (q1|q3), ktd1/ktd2
            kts = {}

            def g_batch(g):
                """Emit G matmuls + EXPs for group g (row-blocks 4g+q).

                Four PE row-tiles run the K=15 matmuls concurrently; each
                [128,1024] PSUM tile pairs two row-blocks (different banks)
                so EXP covers both in one instruction.
                """
                base = {q: 512 * g + 128 * q for q in range(4)}
                if g == 0:
                    # all pair-A chunks first: they only need quadrants
                    # q0/q2, which the DMA order above lands first
                    chunk_order = [(c, p) for p in ("A", "B") for c in range(4)]
                else:
                    chunk_order = [(c, p) for c in range(4) for p in ("A", "B")]
                for c, pair in chunk_order:
                    qa, qb = (0, 2) if pair == "A" else (1, 3)
                    if True:
                        pG = pg.tile([128, 1024], F32, tag="pg")
                        for half, q in ((0, qa), (1, qb)):
                            nc.tensor.matmul(
                                pG[:, 512 * half : 512 * half + 512],
                                pslice(abf, 32 * q, 15, 128 * g, 128 * g + 128),
                                pslice(
                                    bbf,
                                    32 * q,
                                    15,
                                    base[q] + 512 * c,
                                    base[q] + 512 * c + 512,
                                ),
                                start=True,
                                stop=True,
                                tile_position=(32 * q, 0),
                            )
                        kt = kp.tile([128, 1024], BF16, tag="k")
                        nc.scalar.activation(kt[:], pG[:], EXP, bias=bias0[:])
                        kts[(g, pair, c)] = kt
                # d16 (128 cols per row-block), two per PSUM tile in
                # different banks; one strided EXP per tile.
                for pair, (qa, qb) in (("A", (0, 1)), ("B", (2, 3))):
                    pG = pg.tile([128, 1024], F32, tag="pg")
                    for half, q in ((0, qa), (1, qb)):
                        nc.tensor.matmul(
                            pG[:, 512 * half : 512 * half + 128],
                            pslice(abf, 32 * q, 15, 128 * g, 128 * g + 128),
                            pslice(bbf, 32 * q, 15, base[q] + 2048, base[q] + 2176),
                            start=True,
                            stop=True,
                            tile_position=(32 * q, 0),
                        )
                    kt = kp.tile([128, 1024], BF16, tag="k")
                    nc.scalar.activation(
                        kt[:].rearrange("p (a b) -> p a b", b=128)[:, 0:2, :],
                        pG[:].rearrange("p (a b) -> p a b", b=512)[:, :, 0:128],
                        EXP,
                        bias=bias0[:],
                    )
                    kts[(g, "d" + pair, 0)] = kt

            def as_batch(g):
                """Emit AS (4-band column-tiled) + fused multiply-reduce."""
                # band q rhs: q0/q2 from ktA halves, q1/q3 from ktB halves
                src = {0: ("A", 0), 2: ("A", 1), 1: ("B", 0), 3: ("B", 1)}
                for c in range(4):
                    stk = stkp.tile([128, 512], F32, tag="stk")
                    for q in range(4):
                        pair, half = src[q]
                        kt = kts[(g, pair, c)]
                        nc.tensor.matmul(
                            pslice(stk, 32 * q, 32, 0, 512),
                            segT[:, 32 * (4 * g + q) : 32 * (4 * g + q) + 32],
                            kt[:, 512 * half : 512 * half + 512],
                            start=True,
                            stop=True,
                            tile_position=(0, 32 * q),
                        )
                    sct = scp.tile([128, 512], F32, tag="sc")
                    if c == 0:
                        # cols 0:128 are the diagonal d=0 block: weight 1/2
                        nc.vector.scalar_tensor_tensor(
                            out=sct[:, 0:128],
                            in0=stk[:, 0:128],
                            scalar=0.5,
                            in1=segstk[g][:, 0:128],
                            op0=MULT,
                            op1=MULT,
                            accum_out=accT[:, 6 * g + 5 : 6 * g + 6],
                        )
                        nc.vector.scalar_tensor_tensor(
                            out=sct[:, 128:512],
                            in0=stk[:, 128:512],
                            scalar=1.0,
                            in1=segstk[g][:, 128:512],
                            op0=MULT,
                            op1=MULT,
                            accum_out=accT[:, 6 * g : 6 * g + 1],
                        )
                    else:
                        nc.vector.scalar_tensor_tensor(
                            out=sct[:],
                            in0=stk[:],
                            scalar=1.0,
                            in1=segstk[g][:, 512 * c : 512 * c + 512],
                            op0=MULT,
                            op1=MULT,
                            accum_out=accT[:, 6 * g + c : 6 * g + c + 1],
                        )
                # d16: bands q0,q1 from ktdA (cols 0/128), q2,q3 from ktdB
                stk = stkp.tile([128, 512], F32, tag="stk")
                for q in range(4):
                    kt = kts[(g, "dA" if q < 2 else "dB", 0)]
                    nc.tensor.matmul(
                        pslice(stk, 32 * q, 32, 0, 128),
                        segT[:, 32 * (4 * g + q) : 32 * (4 * g + q) + 32],
                        kt[:, 128 * (q % 2) : 128 * (q % 2) + 128],
                        start=True,
                        stop=True,
                        tile_position=(0, 32 * q),
                    )
                sct = scp.tile([128, 512], F32, tag="sc")
                nc.vector.scalar_tensor_tensor(
                    out=sct[:, 0:128],
                    in0=stk[:, 0:128],
                    scalar=0.5,
                    in1=segstk[g][:, 2048:2176],
                    op0=MULT,
                    op1=MULT,
                    accum_out=accT[:, 6 * g + 4 : 6 * g + 5],
                )

            g_batch(0)
            g_batch(1)

            # segT transposes slot in here: seg_sb is ready by the time the
            # PE drains the G(1) tiles, so the in-order PE queue barely
            # stalls and ACT keeps streaming.
            tpps = stkp.tile([128, 512], BF16, tag="stk")
            for v in range(16):
                nc.tensor.transpose(
                    tpps[:, 32 * v : 32 * v + 21],
                    seg_sb[:, 128 * v : 128 * v + 128],
                    ident[:21, :21],
                )
            tp3 = tpps[:].rearrange("p (v c) -> p v c", c=32)
            sg3 = segT[:].rearrange("p (v c) -> p v c", c=32)
            nc.vector.tensor_copy(sg3[:, :, 0:21], tp3[:, :, 0:21])

            as_batch(0)
            g_batch(2)
            as_batch(1)
            g_batch(3)
            as_batch(2)
            as_batch(3)

            nc.sync.dma_start(out_d.ap(), accT[:])
    _split_multiwait(nc)
    return nc


def _split_multiwait(nc):
    """The walrus build here encodes at most one semaphore wait per
    instruction (setupSyncWait: 'Too many sync wait commands'). Tile emits
    multi-wait instructions, so hoist all but one wait onto standalone
    EventSemaphore instructions (what raw-bass wait_ge emits) placed just
    before the instruction on the same engine queue. Semantics identical:
    the engine blocks on each wait in turn."""
    ctr = 0
    for f in nc.m.functions:
        for blk in f.blocks:
            insts = blk.instructions
            out = []
            for inst in insts:
                si = inst.sync_info
                if si is not None and len(si.on_wait) > 1:
                    waits = list(si.on_wait)
                    for w in waits[:-1]:
                        es = mybir.InstEventSemaphore(
                            name=f"WSPLIT-{ctr}", ins=[], outs=[]
                        )
                        ctr += 1
                        es.engine = inst.engine
                        es.sync_info = bass_rust.SyncInfo(on_wait=[w], on_update=[])
                        out.append(es)
                    inst.sync_info = bass_rust.SyncInfo(
                        on_wait=[waits[-1]], on_update=list(si.on_update)
                    )
                out.append(inst)
            insts[:] = out


def _host_prep(images, segmentations):
    """Per-core inputs. Host work is reindexing (roll/stride/reshape) plus
    the tiny [5,4096] feature augmentation; all seg arithmetic happens
    on-device."""
    bf = ml_dtypes.bfloat16
    ident = np.eye(32, dtype=bf)
    in_maps = []
    for c in range(8):
        n, h = c // 2, c % 2
        img = images[n][:, ::2, ::2]                       # nearest resize
        img = np.roll(img, -32 * h, axis=1).reshape(3, P)  # circulant rotation
        f = (img / SIGMA_RGB).astype(np.float32)
        f = f - f.mean(axis=1, keepdims=True)              # d2-invariant centering
        sq = (f * f).sum(axis=0)
        ones = np.ones((1, P), np.float32)
        b5 = np.concatenate([f, ones, (-0.5 * sq)[None]], axis=0)
        a5 = np.concatenate([f, (-0.5 * sq)[None], ones], axis=0)[:, : P // 2]

        def split(x):
            hi = x.astype(bf)
            lo = (x - hi.astype(np.float32)).astype(bf)
            return hi, lo

        a5h, a5l = split(a5)
        b5h, b5l = split(b5)
        a15 = np.concatenate([a5h, a5l, a5h], axis=0)      # [15, 2048] bf16
        b15 = np.concatenate([b5h, b5h, b5l], axis=0)      # [15, 4096] bf16
        b60 = np.ascontiguousarray(np.tile(b15, (4, 1)))   # [60, 4096] bf16
        # abf [60, 512]: rows 15q..15q+14 (-> SBUF quadrant 32q) hold a15 of
        # row-blocks v=4g+q at cols 128g..128g+127 (point 512g+128q+j).
        abf = np.zeros((60, 512), dtype=bf)
        for q in range(4):
            for g in range(4):
                v = 4 * g + q
                abf[15 * q : 15 * q + 15, 128 * g : 128 * g + 128] = a15[
                    :, 128 * v : 128 * v + 128
                ]
        segr = np.roll(segmentations[n], -64 * h, axis=1)  # [21,128,128] f32
        in_maps.append(
            {
                "segr": np.ascontiguousarray(segr, dtype=np.float32),
                "abf": abf,
                "bbf": b60,
                "ident": ident,
            }
        )
    return in_maps


def run(images, segmentations, trace=False):
    if "nc" not in _CACHED:
        _CACHED["nc"] = _build_nc()
    nc = _CACHED["nc"]
    in_maps = _host_prep(np.asarray(images), np.asarray(segmentations))
    res = run_bass_kernel_spmd(nc, in_maps, list(range(8)), trace=trace)
    total = np.float64(0.0)
    for r in res.results:
        total += r["acc"].astype(np.float64).sum()
    # x2 symmetric halves, /16 unscaled 2x2 pool (quadratic), -W, /N batch mean
    loss = -WEIGHT * 2.0 * total / 16.0 / N_IMG
    return np.array([loss], dtype=np.float32), res


def kernel(images, segmentations):
    out, _ = run(images, segmentations, trace=False)
    return out

